# revision 48
# baseline (speedup 1.0000x reference)
"""Causal multi-head attention (B=2, T=4096, C=768, H=12) on 8 Trainium2 cores.

Sharding: core c handles batch b=c//4 and heads 3*(c%4)..3*(c%4)+2 for the
QKV projections and flash attention; one 8-way AllToAll PER HEAD redistributes
that head's attention output so core j holds ALL heads for tq strip j (both
batches), then each core runs the Wo projection for its 2x512 output rows.
The per-head exchanges fire as each head finishes and hide under the next
head's attention; phase 3 consumes the head-major gathered layout against
host-permuted Wo rows (the contraction is order-invariant).

All matmuls run as float32r (tf32-class, full PE rate at free-dim >= 256).
Flash attention uses no-max-subtraction softmax (scores are O(+-5) here, exp
is safe in fp32) with the denominator computed by an appended ones-column on V
(output free-dim 65 = 64 dims + rowsum). Strips 0-3 run strips-outer so all
three heads' exp work fills ACT during the projection-heavy ramp; strips 4-7
run heads-outer so each AllToAll fires early.
"""
import numpy as np
from contextlib import ExitStack

import concourse.bass as bass
import concourse.mybir as mybir
import concourse.tile as tile
from concourse import bacc
from concourse.bass_utils import run_bass_kernel_spmd
from concourse.masks import make_identity, make_upper_triangular

T = 4096
C = 768
H = 12
D = 64
HPC = 3            # heads per core
MPC = HPC * D      # 192 projected dims per core
NCORES = 8
NTB = T // 128     # 32 tk blocks
NQB = T // 512     # 8 tq strips
CB = C // 128      # 6 contraction blocks
f32 = mybir.dt.float32
f32r = mybir.dt.float32r
EXP = mybir.ActivationFunctionType.Exp

_CACHE = {}


def _build():
    nc = bacc.Bacc(None, target_bir_lowering=False, num_devices=NCORES)
    x_in = nc.declare_dram_parameter("x", [T, C], f32r, isOutput=False)
    # weight params typed float32r: the PE rounds f32 operands to f32r
    # internally anyway, so binding raw f32 bits is value-preserving while
    # letting DMA feed matmuls directly (no on-chip rounding copies).
    wq_in = nc.declare_dram_parameter("wq", [C, MPC], f32r, isOutput=False)
    wk_in = nc.declare_dram_parameter("wk", [C, MPC], f32r, isOutput=False)
    wv_in = nc.declare_dram_parameter("wv", [C, MPC], f32r, isOutput=False)
    bq_in = nc.declare_dram_parameter("bq", [MPC], f32, isOutput=False)
    bk_in = nc.declare_dram_parameter("bk", [MPC], f32, isOutput=False)
    bv_in = nc.declare_dram_parameter("bv", [MPC], f32, isOutput=False)
    wo_in = nc.declare_dram_parameter("wo", [C, C], f32r, isOutput=False)
    bo_in = nc.declare_dram_parameter("bo", [C], f32, isOutput=False)
    out_d = nc.declare_dram_parameter("out", [2, 512, C], f32, isOutput=True)

    with tile.TileContext(nc) as tc, ExitStack() as ctx:
        singles = ctx.enter_context(tc.tile_pool(name="singles", bufs=1))
        dram = ctx.enter_context(tc.tile_pool(name="dram", bufs=1, space="DRAM"))

        # ---- static tiles -------------------------------------------------
        # identity in f32r: transpose-mode matmuls then run 1.5 cyc/row vs 2.0
        identity = singles.tile([128, 128], f32r)
        # mask[:, 0:128] = 0, mask[:, 128:256] = upper-tri (c >= r)
        mask = singles.tile([128, 256], f32)
        nc.gpsimd.memset(mask[:, 0:128], 0.0)
        make_upper_triangular(nc, mask[:, 128:256], val=1.0)

        # ---- weights -> SBUF (f32r params: straight DMA, no rounding copies)
        wq_r = singles.tile([128, CB, MPC], f32r)
        wk_r = singles.tile([128, CB, MPC], f32r)
        # wv padded to 256 free cols (zeros) so the v-proj matmul has N=256
        wv_r = singles.tile([128, CB, 256], f32r)
        wo_r = singles.tile([128, CB, C], f32r)
        # weight loads ride SWDGE (gpsimd) so they don't queue ahead of the
        # first x-strip loads on the HWDGE (sync) queues
        nc.gpsimd.dma_start(out=wq_r, in_=wq_in.rearrange("(cb p) m -> p cb m", p=128))
        nc.gpsimd.dma_start(out=wk_r, in_=wk_in.rearrange("(cb p) m -> p cb m", p=128))
        nc.gpsimd.dma_start(
            out=wv_r[:, :, 0:MPC], in_=wv_in.rearrange("(cb p) m -> p cb m", p=128)
        )
        # combined q-tail/k-tail weight: one [128, 512] projection matmul set
        # yields q2 rows 0-63 and k2 rows 64-127
        wqk_t = singles.tile([128, CB, 128], f32r)
        nc.gpsimd.dma_start(
            out=wqk_t[:, :, 0:64],
            in_=wq_in.rearrange("(cb p) m -> p cb m", p=128)[:, :, 128:MPC],
        )
        nc.gpsimd.dma_start(
            out=wqk_t[:, :, 64:128],
            in_=wk_in.rearrange("(cb p) m -> p cb m", p=128)[:, :, 128:MPC],
        )
        with tc.tile_pool(name="wstage", bufs=1) as wstage:
            zpad = wstage.tile([128, CB, 64], f32)
            nc.vector.memset(zpad, 0.0)
            nc.vector.tensor_copy(wv_r[:, :, MPC:256], zpad)
            idf = wstage.tile([128, 128], f32)
            make_identity(nc, idf)
            nc.vector.tensor_copy(identity, idf)
            # pre-trigger the exp table load so its ~2.7us hides in the ramp
            warm = wstage.tile([1, 2], f32)
            nc.vector.memset(warm, 0.0)
            nc.scalar.activation(warm[:, 1:2], warm[:, 0:1], EXP, scale=1.0)

        # ---- biases -------------------------------------------------------
        bq_c = singles.tile([128, 1], f32)
        bk_c = singles.tile([128, 1], f32)
        bq_c2 = singles.tile([64, 1], f32)
        bk_c2h = singles.tile([128, 1], f32)  # k-tail bias parked at rows 64-127
        nc.sync.dma_start(out=bq_c, in_=bq_in[0:128].unsqueeze(1))
        nc.sync.dma_start(out=bk_c, in_=bk_in[0:128].unsqueeze(1))
        nc.sync.dma_start(out=bq_c2, in_=bq_in[128:MPC].unsqueeze(1))
        nc.sync.dma_start(out=bk_c2h[64:128, :], in_=bk_in[128:MPC].unsqueeze(1))
        bv_b = singles.tile([128, MPC], f32)
        nc.sync.dma_start(
            out=bv_b,
            in_=bass.AP(tensor=bv_in.ap().tensor, offset=0, ap=[[0, 128]] + bv_in.ap().ap),
        )
        bo_b = singles.tile([128, C], f32)

        # ---- persistent activation buffers --------------------------------
        # qT/kT per head, d on partitions: heads 0,1 packed into [128, T]
        q01 = singles.tile([128, T], f32r)
        k01 = singles.tile([128, T], f32r)
        q2 = singles.tile([64, T], f32r)
        k2 = singles.tile([64, T], f32r)
        # V + ones column, per tk block and head: [128, 32, 3, 65]
        v1 = singles.tile([128, NTB, HPC, D + 1], f32r)
        ones_t = singles.tile([128, NTB, HPC], f32)
        nc.vector.memset(ones_t, 1.0)
        nc.vector.tensor_copy(v1[:, :, :, D], ones_t)

        # one tile pair per head: head h's AllToAll fires as soon as that
        # head's attention finishes, hiding under the next head's compute
        a2a_in = tuple(
            dram.tile([NCORES, D, 512], f32r, name=f"a2a_in{h}") for h in range(HPC)
        )
        a2a_out = tuple(
            dram.tile([NCORES, D, 512], f32r, name=f"a2a_out{h}") for h in range(HPC)
        )

        # ---- main loop ----------------------------------------------------
        # Strips 0-3 run strips-outer (all heads per strip) so exp work fills
        # ACT during the projection-heavy ramp; strips 4-7 run heads-outer so
        # each head's AllToAll fires early and hides under the next head's
        # attention (projections for strips 4-7 ride along head 0's pass).
        with (
            tc.tile_pool(name="pm", bufs=1) as pm,
            tc.tile_pool(name="psm", bufs=1, space="PSUM") as psm,
            tc.tile_pool(name="drm", bufs=1, space="DRAM") as drm,
        ):
            def do_proj(it):
                xT = pm.tile([128, CB, 512], f32r, tag="xT", bufs=2, name="xT")
                xns = []
                for hf in range(2):
                    xn = pm.tile([128, 2, C], f32r, tag="xn", bufs=3, name="xn")
                    nc.sync.dma_start(
                        out=xn,
                        in_=x_in[
                            512 * it + 256 * hf : 512 * it + 256 * (hf + 1), :
                        ].rearrange("(tb p) c -> p tb c", p=128),
                    )
                    xns.append(xn)
                for cb in range(CB):
                    ps_t = psm.tile([128, 512], f32r, tag="proj", bufs=2, name="ps_t")
                    for hf in range(2):
                        for tb in range(2):
                            nc.tensor.transpose(
                                ps_t[:, 256 * hf + 128 * tb : 256 * hf + 128 * (tb + 1)],
                                xns[hf][:, tb, 128 * cb : 128 * (cb + 1)],
                                identity,
                            )
                    nc.vector.tensor_copy(xT[:, cb, :], ps_t)
                for w_r, bc, dA in ((wq_r, bq_c, q01), (wk_r, bk_c, k01)):
                    psA = psm.tile([128, 512], f32, tag="proj", bufs=2, name="psA")
                    for cb in range(CB):
                        nc.tensor.matmul(
                            psA, w_r[:, cb, 0:128], xT[:, cb, :],
                            start=(cb == 0), stop=(cb == CB - 1),
                        )
                    nc.vector.tensor_scalar_add(
                        dA[:, 512 * it : 512 * (it + 1)], psA, bc
                    )
                # q-tail (head 2 q, rows 0-63) + k-tail (head 2 k, rows 64-127)
                # in one accumulation; k half realigned to base 0 via DMA
                psB = psm.tile([128, 512], f32, tag="proj", bufs=2, name="psB")
                for cb in range(CB):
                    nc.tensor.matmul(
                        psB, wqk_t[:, cb, :], xT[:, cb, :],
                        start=(cb == 0), stop=(cb == CB - 1),
                    )
                nc.vector.tensor_scalar_add(
                    q2[:, 512 * it : 512 * (it + 1)], psB[0:64, :], bq_c2
                )
                ktmp = pm.tile([128, 512], f32r, tag="ktmp", bufs=2, name="ktmp")
                nc.vector.tensor_scalar_add(
                    ktmp[64:128, :], psB[64:128, :], bk_c2h[64:128, :]
                )
                nc.sync.dma_start(
                    out=k2[:, 512 * it : 512 * (it + 1)], in_=ktmp[64:128, :]
                )
                for tb in range(4):
                    psV = psm.tile([128, 256], f32, tag="proj", bufs=2, name="psV")
                    for cb in range(CB):
                        nc.tensor.matmul(
                            psV, xT[:, cb, 128 * tb : 128 * (tb + 1)], wv_r[:, cb, :],
                            start=(cb == 0), stop=(cb == CB - 1),
                        )
                    tk = 4 * it + tb
                    nc.vector.tensor_add(
                        v1[:, tk, :, 0:D],
                        psV[:, 0:MPC].rearrange("p (h d) -> p h d", h=HPC),
                        bv_b.rearrange("p (h d) -> p h d", h=HPC),
                    )

            def do_attn(h, iq):
                qh = (q01[0:64], q01[64:128], q2[0:64])[h]
                kh = (k01[0:64], k01[64:128], k2[0:64])[h]
                ps_o = psm.tile([65, 512], f32, tag="o", bufs=2, name="ps_o")
                qs = qh[:, 512 * iq : 512 * (iq + 1)]
                # full tk blocks in pairs: one [128, 1024] exp, no masking
                for p in range(2 * iq):
                    ik0, ik1 = 2 * p, 2 * p + 1
                    ps2 = psm.tile([128, 1024], f32, tag="s", bufs=2, name="ps2")
                    nc.tensor.matmul(
                        ps2[:, 0:512], kh[:, 128 * ik0 : 128 * (ik0 + 1)], qs,
                        start=True, stop=True,
                    )
                    nc.tensor.matmul(
                        ps2[:, 512:1024], kh[:, 128 * ik1 : 128 * (ik1 + 1)], qs,
                        start=True, stop=True,
                    )
                    pT = pm.tile([128, 1024], f32r, tag="pT", bufs=3, name="pT")
                    nc.scalar.activation(pT, ps2, EXP, scale=0.125)
                    nc.tensor.matmul(
                        ps_o, v1[:, ik0, h, :], pT[:, 0:512],
                        start=(ik0 == 0), stop=False,
                    )
                    nc.tensor.matmul(
                        ps_o, v1[:, ik1, h, :], pT[:, 512:1024],
                        start=False, stop=False,
                    )
                # diagonal region: 4 single blocks with causal masking
                for j in range(4):
                    ik = 4 * iq + j
                    col0 = 0 if j < 1 else (128 if j == 1 else 256)
                    ps2 = psm.tile([128, 1024], f32, tag="s", bufs=2, name="ps2")
                    nc.tensor.matmul(
                        ps2[:, col0:512],
                        kh[:, 128 * ik : 128 * (ik + 1)],
                        qh[:, 512 * iq + col0 : 512 * (iq + 1)],
                        start=True, stop=True,
                    )
                    pT = pm.tile([128, 1024], f32r, tag="pT", bufs=3, name="pT")
                    nc.scalar.activation(pT[:, col0:512], ps2[:, col0:512], EXP, scale=0.125)
                    if j == 3:
                        nc.vector.tensor_mul(pT[:, 256:512], pT[:, 256:512], mask)
                    else:
                        nc.vector.tensor_mul(
                            pT[:, col0 : col0 + 128],
                            pT[:, col0 : col0 + 128],
                            mask[:, 128:256],
                        )
                    nc.tensor.matmul(
                        ps_o[:, col0:], v1[:, ik, h, :], pT[:, col0:512],
                        start=(ik == 0), stop=(j == 3),
                    )
                recip = pm.tile([128, 512], f32, tag="rc", bufs=3, name="recip")
                nc.vector.reciprocal(recip[64:65, :], ps_o[64:65, :])
                rc_d = drm.tile([512], f32, tag="rcd", bufs=3, name="rc_d")
                nc.sync.dma_start(out=rc_d.unsqueeze(0), in_=recip[64:65, :])
                bcast = pm.tile([64, 512], f32, tag="bc", bufs=3, name="bcast")
                nc.sync.dma_start(
                    out=bcast,
                    in_=bass.AP(tensor=rc_d.tensor, offset=rc_d[:].offset, ap=[[0, 64]] + rc_d[:].ap),
                )
                att_n = pm.tile([64, 512], f32r, tag="an", bufs=3, name="att_n")
                nc.vector.tensor_mul(att_n, ps_o[0:64, :], bcast)
                nc.sync.dma_start(out=a2a_in[h][iq, :, :], in_=att_n)

            for iq in range(4):
                do_proj(iq)
                for h in range(HPC):
                    do_attn(h, iq)
            for h in range(HPC):
                for iq in range(4, NQB):
                    if h == 0:
                        do_proj(iq)
                    do_attn(h, iq)
                nc.gpsimd.collective_compute(
                    "AllToAll",
                    mybir.AluOpType.bypass,
                    replica_groups=[list(range(NCORES))],
                    ins=[a2a_in[h][:]],
                    outs=[a2a_out[h][:]],
                )

        # ---- phase 3: output projection -----------------------------------
        # gathered layout is head-major: flats[h] rows = 64*src + d; the host
        # permutes Wo's rows to match (see kernel()).
        flats = tuple(a.rearrange("s d t -> (s d) t") for a in a2a_out)  # [512, 512]
        with (
            tc.tile_pool(name="p3", bufs=1) as p3,
            tc.tile_pool(name="ps3", bufs=1, space="PSUM") as ps3,
        ):
            nc.sync.dma_start(out=wo_r, in_=wo_in.rearrange("(cb p) m -> p cb m", p=128))
            nc.sync.dma_start(
                out=bo_b,
                in_=bass.AP(tensor=bo_in.ap().tensor, offset=0, ap=[[0, 128]] + bo_in.ap().ap),
            )
            for bb in range(2):
                for tb in range(4):
                    ps_a = ps3.tile([128, 512], f32, tag="a", bufs=4)
                    ps_b = ps3.tile([128, 256], f32, tag="b", bufs=4)
                    for idx in range(CB):
                        h_l, half = divmod(idx, 2)
                        lt = p3.tile([128, 128], f32r, tag="ltr", bufs=12)
                        nc.sync.dma_start(
                            out=lt,
                            in_=flats[h_l][
                                256 * bb + 128 * half : 256 * bb + 128 * (half + 1),
                                128 * tb : 128 * (tb + 1),
                            ],
                        )
                        nc.tensor.matmul(
                            ps_a, lt, wo_r[:, idx, 0:512],
                            start=(idx == 0), stop=(idx == CB - 1),
                        )
                        nc.tensor.matmul(
                            ps_b, lt, wo_r[:, idx, 512:C],
                            start=(idx == 0), stop=(idx == CB - 1),
                        )
                    out_t = p3.tile([128, C], f32, tag="ot", bufs=3)
                    nc.vector.tensor_add(out_t[:, 0:512], ps_a, bo_b[:, 0:512])
                    nc.vector.tensor_add(out_t[:, 512:C], ps_b, bo_b[:, 512:C])
                    nc.sync.dma_start(
                        out=out_d[bb, 128 * tb : 128 * (tb + 1), :], in_=out_t
                    )

    nc.finalize()
    return nc


def kernel(x, Wq, bq, Wk, bk, Wv, bv, Wo, bo):
    if "nc" not in _CACHE:
        _CACHE["nc"] = _build()
    nc = _CACHE["nc"]

    x = np.asarray(x, dtype=np.float32)
    # permute Wo rows from global head-dim order (192g + 64h + d) to the
    # head-major gathered layout (256h + 64g + d) used by phase 3
    perm = np.empty(C, dtype=np.int64)
    for h_l in range(HPC):
        for g in range(4):
            perm[256 * h_l + 64 * g : 256 * h_l + 64 * g + 64] = np.arange(
                MPC * g + D * h_l, MPC * g + D * h_l + D
            )
    wo_send = np.ascontiguousarray(np.asarray(Wo, np.float32)[perm, :])
    in_maps = []
    for c in range(NCORES):
        b, g = c // 4, c % 4
        sl = slice(MPC * g, MPC * (g + 1))
        in_maps.append({
            "x": np.ascontiguousarray(x[b]),
            "wq": np.ascontiguousarray(np.asarray(Wq, np.float32)[:, sl]),
            "wk": np.ascontiguousarray(np.asarray(Wk, np.float32)[:, sl]),
            "wv": np.ascontiguousarray(np.asarray(Wv, np.float32)[:, sl]),
            "bq": np.ascontiguousarray(np.asarray(bq, np.float32)[sl]),
            "bk": np.ascontiguousarray(np.asarray(bk, np.float32)[sl]),
            "bv": np.ascontiguousarray(np.asarray(bv, np.float32)[sl]),
            "wo": wo_send,
            "bo": np.ascontiguousarray(np.asarray(bo, np.float32)),
        })

    res = run_bass_kernel_spmd(nc, in_maps, core_ids=list(range(NCORES)))
    out = np.empty((2, T, C), dtype=np.float32)
    for j in range(NCORES):
        r = res.results[j]["out"]
        out[0, 512 * j : 512 * (j + 1), :] = r[0]
        out[1, 512 * j : 512 * (j + 1), :] = r[1]
    return out


# revision 51
# speedup vs baseline: 1.0017x; 1.0017x over previous
"""Causal multi-head attention (B=2, T=4096, C=768, H=12) on 8 Trainium2 cores.

Sharding: core c handles batch b=c//4 and heads 3*(c%4)..3*(c%4)+2 for the
QKV projections and flash attention; one 8-way AllToAll PER HEAD redistributes
that head's attention output so core j holds ALL heads for tq strip j (both
batches), then each core runs the Wo projection for its 2x512 output rows.
The per-head exchanges fire as each head finishes and hide under the next
head's attention; phase 3 consumes the head-major gathered layout against
host-permuted Wo rows (the contraction is order-invariant).

All matmuls run as float32r (tf32-class, full PE rate at free-dim >= 256).
Flash attention uses no-max-subtraction softmax (scores are O(+-5) here, exp
is safe in fp32) with the denominator computed by an appended ones-column on V
(output free-dim 65 = 64 dims + rowsum). Strips 0-3 run strips-outer so all
three heads' exp work fills ACT during the projection-heavy ramp; strips 4-7
run heads-outer so each AllToAll fires early.
"""
import numpy as np
from contextlib import ExitStack

import concourse.bass as bass
import concourse.mybir as mybir
import concourse.tile as tile
from concourse import bacc
from concourse.bass_utils import run_bass_kernel_spmd
from concourse.masks import make_identity, make_upper_triangular

T = 4096
C = 768
H = 12
D = 64
HPC = 3            # heads per core
MPC = HPC * D      # 192 projected dims per core
NCORES = 8
NTB = T // 128     # 32 tk blocks
NQB = T // 512     # 8 tq strips
CB = C // 128      # 6 contraction blocks
f32 = mybir.dt.float32
f32r = mybir.dt.float32r
EXP = mybir.ActivationFunctionType.Exp

_CACHE = {}


def _build():
    nc = bacc.Bacc(None, target_bir_lowering=False, num_devices=NCORES)
    x_in = nc.declare_dram_parameter("x", [T, C], f32r, isOutput=False)
    # weight params typed float32r: the PE rounds f32 operands to f32r
    # internally anyway, so binding raw f32 bits is value-preserving while
    # letting DMA feed matmuls directly (no on-chip rounding copies).
    wq_in = nc.declare_dram_parameter("wq", [C, MPC], f32r, isOutput=False)
    wk_in = nc.declare_dram_parameter("wk", [C, MPC], f32r, isOutput=False)
    wv_in = nc.declare_dram_parameter("wv", [C, MPC], f32r, isOutput=False)
    bq_in = nc.declare_dram_parameter("bq", [MPC], f32, isOutput=False)
    bk_in = nc.declare_dram_parameter("bk", [MPC], f32, isOutput=False)
    bv_in = nc.declare_dram_parameter("bv", [MPC], f32, isOutput=False)
    wo_in = nc.declare_dram_parameter("wo", [C, C], f32r, isOutput=False)
    bo_in = nc.declare_dram_parameter("bo", [C], f32, isOutput=False)
    out_d = nc.declare_dram_parameter("out", [2, 512, C], f32, isOutput=True)

    with tile.TileContext(nc) as tc, ExitStack() as ctx:
        singles = ctx.enter_context(tc.tile_pool(name="singles", bufs=1))
        dram = ctx.enter_context(tc.tile_pool(name="dram", bufs=1, space="DRAM"))

        # ---- static tiles -------------------------------------------------
        # identity in f32r: transpose-mode matmuls then run 1.5 cyc/row vs 2.0
        identity = singles.tile([128, 128], f32r)
        # mask[:, 0:128] = 0, mask[:, 128:256] = upper-tri (c >= r)
        mask = singles.tile([128, 256], f32)
        nc.gpsimd.memset(mask[:, 0:128], 0.0)
        make_upper_triangular(nc, mask[:, 128:256], val=1.0)

        # ---- weights -> SBUF (f32r params: straight DMA, no rounding copies)
        wq_r = singles.tile([128, CB, MPC], f32r)
        wk_r = singles.tile([128, CB, MPC], f32r)
        # wv padded to 256 free cols (zeros) so the v-proj matmul has N=256
        wv_r = singles.tile([128, CB, 256], f32r)
        wo_r = singles.tile([128, CB, C], f32r)
        # weight loads ride SWDGE (gpsimd) so they don't queue ahead of the
        # first x-strip loads on the HWDGE (sync) queues
        nc.gpsimd.dma_start(out=wq_r, in_=wq_in.rearrange("(cb p) m -> p cb m", p=128))
        nc.gpsimd.dma_start(out=wk_r, in_=wk_in.rearrange("(cb p) m -> p cb m", p=128))
        nc.gpsimd.dma_start(
            out=wv_r[:, :, 0:MPC], in_=wv_in.rearrange("(cb p) m -> p cb m", p=128)
        )
        # combined q-tail/k-tail weight: one [128, 512] projection matmul set
        # yields q2 rows 0-63 and k2 rows 64-127
        wqk_t = singles.tile([128, CB, 128], f32r)
        nc.gpsimd.dma_start(
            out=wqk_t[:, :, 0:64],
            in_=wq_in.rearrange("(cb p) m -> p cb m", p=128)[:, :, 128:MPC],
        )
        nc.gpsimd.dma_start(
            out=wqk_t[:, :, 64:128],
            in_=wk_in.rearrange("(cb p) m -> p cb m", p=128)[:, :, 128:MPC],
        )
        with tc.tile_pool(name="wstage", bufs=1) as wstage:
            zpad = wstage.tile([128, CB, 64], f32)
            nc.vector.memset(zpad, 0.0)
            nc.vector.tensor_copy(wv_r[:, :, MPC:256], zpad)
            idf = wstage.tile([128, 128], f32)
            make_identity(nc, idf)
            nc.vector.tensor_copy(identity, idf)
            # pre-trigger the exp table load so its ~2.7us hides in the ramp
            warm = wstage.tile([1, 2], f32)
            nc.vector.memset(warm, 0.0)
            nc.scalar.activation(warm[:, 1:2], warm[:, 0:1], EXP, scale=1.0)

        # ---- biases -------------------------------------------------------
        bq_c = singles.tile([128, 1], f32)
        bk_c = singles.tile([128, 1], f32)
        bq_c2 = singles.tile([64, 1], f32)
        bk_c2h = singles.tile([128, 1], f32)  # k-tail bias parked at rows 64-127
        nc.gpsimd.dma_start(out=bq_c, in_=bq_in[0:128].unsqueeze(1))
        nc.gpsimd.dma_start(out=bk_c, in_=bk_in[0:128].unsqueeze(1))
        nc.gpsimd.dma_start(out=bq_c2, in_=bq_in[128:MPC].unsqueeze(1))
        nc.gpsimd.dma_start(out=bk_c2h[64:128, :], in_=bk_in[128:MPC].unsqueeze(1))
        bv_b = singles.tile([128, MPC], f32)
        nc.gpsimd.dma_start(
            out=bv_b,
            in_=bass.AP(tensor=bv_in.ap().tensor, offset=0, ap=[[0, 128]] + bv_in.ap().ap),
        )
        bo_b = singles.tile([128, C], f32)

        # ---- persistent activation buffers --------------------------------
        # qT/kT per head, d on partitions: heads 0,1 packed into [128, T]
        q01 = singles.tile([128, T], f32r)
        k01 = singles.tile([128, T], f32r)
        q2 = singles.tile([64, T], f32r)
        k2 = singles.tile([64, T], f32r)
        # V + ones column, per tk block and head: [128, 32, 3, 65]
        v1 = singles.tile([128, NTB, HPC, D + 1], f32r)
        ones_t = singles.tile([128, NTB, HPC], f32)
        nc.vector.memset(ones_t, 1.0)
        nc.vector.tensor_copy(v1[:, :, :, D], ones_t)

        # one tile pair per head: head h's AllToAll fires as soon as that
        # head's attention finishes, hiding under the next head's compute
        a2a_in = tuple(
            dram.tile([NCORES, D, 512], f32r, name=f"a2a_in{h}") for h in range(HPC)
        )
        a2a_out = tuple(
            dram.tile([NCORES, D, 512], f32r, name=f"a2a_out{h}") for h in range(HPC)
        )

        # ---- main loop ----------------------------------------------------
        # Strips 0-3 run strips-outer (all heads per strip) so exp work fills
        # ACT during the projection-heavy ramp; strips 4-7 run heads-outer so
        # each head's AllToAll fires early and hides under the next head's
        # attention (projections for strips 4-7 ride along head 0's pass).
        with (
            tc.tile_pool(name="pm", bufs=1) as pm,
            tc.tile_pool(name="psm", bufs=1, space="PSUM") as psm,
            tc.tile_pool(name="drm", bufs=1, space="DRAM") as drm,
        ):
            def do_proj(it):
                xT = pm.tile([128, CB, 512], f32r, tag="xT", bufs=2, name="xT")
                xns = []
                for hf in range(2):
                    xn = pm.tile([128, 2, C], f32r, tag="xn", bufs=3, name="xn")
                    nc.sync.dma_start(
                        out=xn,
                        in_=x_in[
                            512 * it + 256 * hf : 512 * it + 256 * (hf + 1), :
                        ].rearrange("(tb p) c -> p tb c", p=128),
                    )
                    xns.append(xn)
                for cb in range(CB):
                    ps_t = psm.tile([128, 512], f32r, tag="proj", bufs=2, name="ps_t")
                    for hf in range(2):
                        for tb in range(2):
                            nc.tensor.transpose(
                                ps_t[:, 256 * hf + 128 * tb : 256 * hf + 128 * (tb + 1)],
                                xns[hf][:, tb, 128 * cb : 128 * (cb + 1)],
                                identity,
                            )
                    nc.vector.tensor_copy(xT[:, cb, :], ps_t)
                for w_r, bc, dA in ((wq_r, bq_c, q01), (wk_r, bk_c, k01)):
                    psA = psm.tile([128, 512], f32, tag="proj", bufs=2, name="psA")
                    for cb in range(CB):
                        nc.tensor.matmul(
                            psA, w_r[:, cb, 0:128], xT[:, cb, :],
                            start=(cb == 0), stop=(cb == CB - 1),
                        )
                    nc.vector.tensor_scalar_add(
                        dA[:, 512 * it : 512 * (it + 1)], psA, bc
                    )
                # q-tail (head 2 q, rows 0-63) + k-tail (head 2 k, rows 64-127)
                # in one accumulation; k half realigned to base 0 via DMA
                psB = psm.tile([128, 512], f32, tag="proj", bufs=2, name="psB")
                for cb in range(CB):
                    nc.tensor.matmul(
                        psB, wqk_t[:, cb, :], xT[:, cb, :],
                        start=(cb == 0), stop=(cb == CB - 1),
                    )
                nc.vector.tensor_scalar_add(
                    q2[:, 512 * it : 512 * (it + 1)], psB[0:64, :], bq_c2
                )
                ktmp = pm.tile([128, 512], f32r, tag="ktmp", bufs=2, name="ktmp")
                nc.vector.tensor_scalar_add(
                    ktmp[64:128, :], psB[64:128, :], bk_c2h[64:128, :]
                )
                nc.sync.dma_start(
                    out=k2[:, 512 * it : 512 * (it + 1)], in_=ktmp[64:128, :]
                )
                for tb in range(4):
                    psV = psm.tile([128, 256], f32, tag="proj", bufs=2, name="psV")
                    for cb in range(CB):
                        nc.tensor.matmul(
                            psV, xT[:, cb, 128 * tb : 128 * (tb + 1)], wv_r[:, cb, :],
                            start=(cb == 0), stop=(cb == CB - 1),
                        )
                    tk = 4 * it + tb
                    nc.vector.tensor_add(
                        v1[:, tk, :, 0:D],
                        psV[:, 0:MPC].rearrange("p (h d) -> p h d", h=HPC),
                        bv_b.rearrange("p (h d) -> p h d", h=HPC),
                    )

            def do_attn(h, iq):
                qh = (q01[0:64], q01[64:128], q2[0:64])[h]
                kh = (k01[0:64], k01[64:128], k2[0:64])[h]
                ps_o = psm.tile([65, 512], f32, tag="o", bufs=2, name="ps_o")
                qs = qh[:, 512 * iq : 512 * (iq + 1)]
                # full tk blocks in pairs: one [128, 1024] exp, no masking
                for p in range(2 * iq):
                    ik0, ik1 = 2 * p, 2 * p + 1
                    ps2 = psm.tile([128, 1024], f32, tag="s", bufs=2, name="ps2")
                    nc.tensor.matmul(
                        ps2[:, 0:512], kh[:, 128 * ik0 : 128 * (ik0 + 1)], qs,
                        start=True, stop=True,
                    )
                    nc.tensor.matmul(
                        ps2[:, 512:1024], kh[:, 128 * ik1 : 128 * (ik1 + 1)], qs,
                        start=True, stop=True,
                    )
                    pT = pm.tile([128, 1024], f32r, tag="pT", bufs=3, name="pT")
                    nc.scalar.activation(pT, ps2, EXP, scale=0.125)
                    nc.tensor.matmul(
                        ps_o, v1[:, ik0, h, :], pT[:, 0:512],
                        start=(ik0 == 0), stop=False,
                    )
                    nc.tensor.matmul(
                        ps_o, v1[:, ik1, h, :], pT[:, 512:1024],
                        start=False, stop=False,
                    )
                # diagonal region: 4 single blocks with causal masking
                for j in range(4):
                    ik = 4 * iq + j
                    col0 = 0 if j < 1 else (128 if j == 1 else 256)
                    ps2 = psm.tile([128, 1024], f32, tag="s", bufs=2, name="ps2")
                    nc.tensor.matmul(
                        ps2[:, col0:512],
                        kh[:, 128 * ik : 128 * (ik + 1)],
                        qh[:, 512 * iq + col0 : 512 * (iq + 1)],
                        start=True, stop=True,
                    )
                    pT = pm.tile([128, 1024], f32r, tag="pT", bufs=3, name="pT")
                    nc.scalar.activation(pT[:, col0:512], ps2[:, col0:512], EXP, scale=0.125)
                    if j == 3:
                        nc.vector.tensor_mul(pT[:, 256:512], pT[:, 256:512], mask)
                    else:
                        nc.vector.tensor_mul(
                            pT[:, col0 : col0 + 128],
                            pT[:, col0 : col0 + 128],
                            mask[:, 128:256],
                        )
                    nc.tensor.matmul(
                        ps_o[:, col0:], v1[:, ik, h, :], pT[:, col0:512],
                        start=(ik == 0), stop=(j == 3),
                    )
                recip = pm.tile([128, 512], f32, tag="rc", bufs=3, name="recip")
                nc.vector.reciprocal(recip[64:65, :], ps_o[64:65, :])
                rc_d = drm.tile([512], f32, tag="rcd", bufs=3, name="rc_d")
                nc.sync.dma_start(out=rc_d.unsqueeze(0), in_=recip[64:65, :])
                bcast = pm.tile([64, 512], f32, tag="bc", bufs=3, name="bcast")
                nc.sync.dma_start(
                    out=bcast,
                    in_=bass.AP(tensor=rc_d.tensor, offset=rc_d[:].offset, ap=[[0, 64]] + rc_d[:].ap),
                )
                att_n = pm.tile([64, 512], f32r, tag="an", bufs=3, name="att_n")
                nc.vector.tensor_mul(att_n, ps_o[0:64, :], bcast)
                nc.sync.dma_start(out=a2a_in[h][iq, :, :], in_=att_n)

            for iq in range(4):
                do_proj(iq)
                for h in range(HPC):
                    do_attn(h, iq)
            for h in range(HPC):
                for iq in range(4, NQB):
                    if h == 0:
                        do_proj(iq)
                    do_attn(h, iq)
                nc.gpsimd.collective_compute(
                    "AllToAll",
                    mybir.AluOpType.bypass,
                    replica_groups=[list(range(NCORES))],
                    ins=[a2a_in[h][:]],
                    outs=[a2a_out[h][:]],
                )

        # ---- phase 3: output projection -----------------------------------
        # gathered layout is head-major: flats[h] rows = 64*src + d; the host
        # permutes Wo's rows to match (see kernel()).
        flats = tuple(a.rearrange("s d t -> (s d) t") for a in a2a_out)  # [512, 512]
        with (
            tc.tile_pool(name="p3", bufs=1) as p3,
            tc.tile_pool(name="ps3", bufs=1, space="PSUM") as ps3,
        ):
            nc.sync.dma_start(out=wo_r, in_=wo_in.rearrange("(cb p) m -> p cb m", p=128))
            nc.sync.dma_start(
                out=bo_b,
                in_=bass.AP(tensor=bo_in.ap().tensor, offset=0, ap=[[0, 128]] + bo_in.ap().ap),
            )
            for bb in range(2):
                for tb in range(4):
                    ps_a = ps3.tile([128, 512], f32, tag="a", bufs=4)
                    ps_b = ps3.tile([128, 256], f32, tag="b", bufs=4)
                    for idx in range(CB):
                        h_l, half = divmod(idx, 2)
                        lt = p3.tile([128, 128], f32r, tag="ltr", bufs=12)
                        nc.sync.dma_start(
                            out=lt,
                            in_=flats[h_l][
                                256 * bb + 128 * half : 256 * bb + 128 * (half + 1),
                                128 * tb : 128 * (tb + 1),
                            ],
                        )
                        nc.tensor.matmul(
                            ps_a, lt, wo_r[:, idx, 0:512],
                            start=(idx == 0), stop=(idx == CB - 1),
                        )
                        nc.tensor.matmul(
                            ps_b, lt, wo_r[:, idx, 512:C],
                            start=(idx == 0), stop=(idx == CB - 1),
                        )
                    out_t = p3.tile([128, C], f32, tag="ot", bufs=3)
                    nc.vector.tensor_add(out_t[:, 0:512], ps_a, bo_b[:, 0:512])
                    nc.vector.tensor_add(out_t[:, 512:C], ps_b, bo_b[:, 512:C])
                    nc.sync.dma_start(
                        out=out_d[bb, 128 * tb : 128 * (tb + 1), :], in_=out_t
                    )

    nc.finalize()
    return nc


def kernel(x, Wq, bq, Wk, bk, Wv, bv, Wo, bo):
    if "nc" not in _CACHE:
        _CACHE["nc"] = _build()
    nc = _CACHE["nc"]

    x = np.asarray(x, dtype=np.float32)
    # permute Wo rows from global head-dim order (192g + 64h + d) to the
    # head-major gathered layout (256h + 64g + d) used by phase 3
    perm = np.empty(C, dtype=np.int64)
    for h_l in range(HPC):
        for g in range(4):
            perm[256 * h_l + 64 * g : 256 * h_l + 64 * g + 64] = np.arange(
                MPC * g + D * h_l, MPC * g + D * h_l + D
            )
    wo_send = np.ascontiguousarray(np.asarray(Wo, np.float32)[perm, :])
    in_maps = []
    for c in range(NCORES):
        b, g = c // 4, c % 4
        sl = slice(MPC * g, MPC * (g + 1))
        in_maps.append({
            "x": np.ascontiguousarray(x[b]),
            "wq": np.ascontiguousarray(np.asarray(Wq, np.float32)[:, sl]),
            "wk": np.ascontiguousarray(np.asarray(Wk, np.float32)[:, sl]),
            "wv": np.ascontiguousarray(np.asarray(Wv, np.float32)[:, sl]),
            "bq": np.ascontiguousarray(np.asarray(bq, np.float32)[sl]),
            "bk": np.ascontiguousarray(np.asarray(bk, np.float32)[sl]),
            "bv": np.ascontiguousarray(np.asarray(bv, np.float32)[sl]),
            "wo": wo_send,
            "bo": np.ascontiguousarray(np.asarray(bo, np.float32)),
        })

    res = run_bass_kernel_spmd(nc, in_maps, core_ids=list(range(NCORES)))
    out = np.empty((2, T, C), dtype=np.float32)
    for j in range(NCORES):
        r = res.results[j]["out"]
        out[0, 512 * j : 512 * (j + 1), :] = r[0]
        out[1, 512 * j : 512 * (j + 1), :] = r[1]
    return out


# revision 52
# speedup vs baseline: 1.0067x; 1.0049x over previous
"""Causal multi-head attention (B=2, T=4096, C=768, H=12) on 8 Trainium2 cores.

Sharding: core c handles batch b=c//4 and heads 3*(c%4)..3*(c%4)+2 for the
QKV projections and flash attention; one 8-way AllToAll PER HEAD redistributes
that head's attention output so core j holds ALL heads for tq strip j (both
batches), then each core runs the Wo projection for its 2x512 output rows.
The per-head exchanges fire as each head finishes and hide under the next
head's attention; phase 3 consumes the head-major gathered layout against
host-permuted Wo rows (the contraction is order-invariant).

All matmuls run as float32r (tf32-class, full PE rate at free-dim >= 256).
Flash attention uses no-max-subtraction softmax (scores are O(+-5) here, exp
is safe in fp32) with the denominator computed by an appended ones-column on V
(output free-dim 65 = 64 dims + rowsum). Strips 0-3 run strips-outer so all
three heads' exp work fills ACT during the projection-heavy ramp; strips 4-7
run heads-outer so each AllToAll fires early.
"""
import numpy as np
from contextlib import ExitStack

import concourse.bass as bass
import concourse.mybir as mybir
import concourse.tile as tile
from concourse import bacc
from concourse.bass_utils import run_bass_kernel_spmd
from concourse.masks import make_identity, make_upper_triangular

T = 4096
C = 768
H = 12
D = 64
HPC = 3            # heads per core
MPC = HPC * D      # 192 projected dims per core
NCORES = 8
NTB = T // 128     # 32 tk blocks
NQB = T // 512     # 8 tq strips
CB = C // 128      # 6 contraction blocks
f32 = mybir.dt.float32
f32r = mybir.dt.float32r
EXP = mybir.ActivationFunctionType.Exp

_CACHE = {}


def _build():
    nc = bacc.Bacc(None, target_bir_lowering=False, num_devices=NCORES)
    x_in = nc.declare_dram_parameter("x", [T, C], f32r, isOutput=False)
    # weight params typed float32r: the PE rounds f32 operands to f32r
    # internally anyway, so binding raw f32 bits is value-preserving while
    # letting DMA feed matmuls directly (no on-chip rounding copies).
    wq_in = nc.declare_dram_parameter("wq", [C, MPC], f32r, isOutput=False)
    wk_in = nc.declare_dram_parameter("wk", [C, MPC], f32r, isOutput=False)
    wv_in = nc.declare_dram_parameter("wv", [C, MPC], f32r, isOutput=False)
    bq_in = nc.declare_dram_parameter("bq", [MPC], f32, isOutput=False)
    bk_in = nc.declare_dram_parameter("bk", [MPC], f32, isOutput=False)
    bv_in = nc.declare_dram_parameter("bv", [MPC], f32, isOutput=False)
    wo_in = nc.declare_dram_parameter("wo", [C, C], f32r, isOutput=False)
    bo_in = nc.declare_dram_parameter("bo", [C], f32, isOutput=False)
    out_d = nc.declare_dram_parameter("out", [2, 512, C], f32, isOutput=True)

    with tile.TileContext(nc) as tc, ExitStack() as ctx:
        singles = ctx.enter_context(tc.tile_pool(name="singles", bufs=1))
        dram = ctx.enter_context(tc.tile_pool(name="dram", bufs=1, space="DRAM"))

        # ---- static tiles -------------------------------------------------
        # identity in f32r: transpose-mode matmuls then run 1.5 cyc/row vs 2.0
        identity = singles.tile([128, 128], f32r)
        # mask[:, 0:128] = 0, mask[:, 128:256] = upper-tri (c >= r)
        mask = singles.tile([128, 256], f32)
        nc.gpsimd.memset(mask[:, 0:128], 0.0)
        make_upper_triangular(nc, mask[:, 128:256], val=1.0)

        # ---- weights -> SBUF (f32r params: straight DMA, no rounding copies)
        wq_r = singles.tile([128, CB, MPC], f32r)
        wk_r = singles.tile([128, CB, MPC], f32r)
        # wv padded to 256 free cols (zeros) so the v-proj matmul has N=256
        wv_r = singles.tile([128, CB, 256], f32r)
        wo_r = singles.tile([128, CB, C], f32r)
        # identity FIRST on the gpsimd queue so the first transpose isn't
        # gated behind the weight DMAs below
        with tc.tile_pool(name="idstage", bufs=1) as idstage:
            idf = idstage.tile([128, 128], f32)
            make_identity(nc, idf)
            nc.vector.tensor_copy(identity, idf)
        # weight loads ride SWDGE (gpsimd) so they don't queue ahead of the
        # first x-strip loads on the HWDGE (sync) queues
        nc.gpsimd.dma_start(out=wq_r, in_=wq_in.rearrange("(cb p) m -> p cb m", p=128))
        nc.gpsimd.dma_start(out=wk_r, in_=wk_in.rearrange("(cb p) m -> p cb m", p=128))
        nc.gpsimd.dma_start(
            out=wv_r[:, :, 0:MPC], in_=wv_in.rearrange("(cb p) m -> p cb m", p=128)
        )
        # combined q-tail/k-tail weight: one [128, 512] projection matmul set
        # yields q2 rows 0-63 and k2 rows 64-127
        wqk_t = singles.tile([128, CB, 128], f32r)
        nc.gpsimd.dma_start(
            out=wqk_t[:, :, 0:64],
            in_=wq_in.rearrange("(cb p) m -> p cb m", p=128)[:, :, 128:MPC],
        )
        nc.gpsimd.dma_start(
            out=wqk_t[:, :, 64:128],
            in_=wk_in.rearrange("(cb p) m -> p cb m", p=128)[:, :, 128:MPC],
        )
        with tc.tile_pool(name="wstage", bufs=1) as wstage:
            zpad = wstage.tile([128, CB, 64], f32)
            nc.vector.memset(zpad, 0.0)
            nc.vector.tensor_copy(wv_r[:, :, MPC:256], zpad)
            # pre-trigger the exp table load so its ~2.7us hides in the ramp
            warm = wstage.tile([1, 2], f32)
            nc.vector.memset(warm, 0.0)
            nc.scalar.activation(warm[:, 1:2], warm[:, 0:1], EXP, scale=1.0)

        # ---- biases -------------------------------------------------------
        bq_c = singles.tile([128, 1], f32)
        bk_c = singles.tile([128, 1], f32)
        bq_c2 = singles.tile([64, 1], f32)
        bk_c2h = singles.tile([128, 1], f32)  # k-tail bias parked at rows 64-127
        nc.gpsimd.dma_start(out=bq_c, in_=bq_in[0:128].unsqueeze(1))
        nc.gpsimd.dma_start(out=bk_c, in_=bk_in[0:128].unsqueeze(1))
        nc.gpsimd.dma_start(out=bq_c2, in_=bq_in[128:MPC].unsqueeze(1))
        nc.gpsimd.dma_start(out=bk_c2h[64:128, :], in_=bk_in[128:MPC].unsqueeze(1))
        bv_b = singles.tile([128, MPC], f32)
        nc.gpsimd.dma_start(
            out=bv_b,
            in_=bass.AP(tensor=bv_in.ap().tensor, offset=0, ap=[[0, 128]] + bv_in.ap().ap),
        )
        bo_b = singles.tile([128, C], f32)

        # ---- persistent activation buffers --------------------------------
        # qT/kT per head, d on partitions: heads 0,1 packed into [128, T]
        q01 = singles.tile([128, T], f32r)
        k01 = singles.tile([128, T], f32r)
        q2 = singles.tile([64, T], f32r)
        k2 = singles.tile([64, T], f32r)
        # V + ones column, per tk block and head: [128, 32, 3, 65]
        v1 = singles.tile([128, NTB, HPC, D + 1], f32r)
        ones_t = singles.tile([128, NTB, HPC], f32)
        nc.vector.memset(ones_t, 1.0)
        nc.vector.tensor_copy(v1[:, :, :, D], ones_t)

        # one tile pair per head: head h's AllToAll fires as soon as that
        # head's attention finishes, hiding under the next head's compute
        a2a_in = tuple(
            dram.tile([NCORES, D, 512], f32r, name=f"a2a_in{h}") for h in range(HPC)
        )
        a2a_out = tuple(
            dram.tile([NCORES, D, 512], f32r, name=f"a2a_out{h}") for h in range(HPC)
        )

        # ---- main loop ----------------------------------------------------
        # Strips 0-3 run strips-outer (all heads per strip) so exp work fills
        # ACT during the projection-heavy ramp; strips 4-7 run heads-outer so
        # each head's AllToAll fires early and hides under the next head's
        # attention (projections for strips 4-7 ride along head 0's pass).
        with (
            tc.tile_pool(name="pm", bufs=1) as pm,
            tc.tile_pool(name="psm", bufs=1, space="PSUM") as psm,
            tc.tile_pool(name="drm", bufs=1, space="DRAM") as drm,
        ):
            def do_proj(it):
                xT = pm.tile([128, CB, 512], f32r, tag="xT", bufs=2, name="xT")
                xns = []
                for hf in range(2):
                    xn = pm.tile([128, 2, C], f32r, tag="xn", bufs=3, name="xn")
                    nc.sync.dma_start(
                        out=xn,
                        in_=x_in[
                            512 * it + 256 * hf : 512 * it + 256 * (hf + 1), :
                        ].rearrange("(tb p) c -> p tb c", p=128),
                    )
                    xns.append(xn)
                for cb in range(CB):
                    ps_t = psm.tile([128, 512], f32r, tag="proj", bufs=2, name="ps_t")
                    for hf in range(2):
                        for tb in range(2):
                            nc.tensor.transpose(
                                ps_t[:, 256 * hf + 128 * tb : 256 * hf + 128 * (tb + 1)],
                                xns[hf][:, tb, 128 * cb : 128 * (cb + 1)],
                                identity,
                            )
                    nc.vector.tensor_copy(xT[:, cb, :], ps_t)
                for w_r, bc, dA in ((wq_r, bq_c, q01), (wk_r, bk_c, k01)):
                    psA = psm.tile([128, 512], f32, tag="proj", bufs=2, name="psA")
                    for cb in range(CB):
                        nc.tensor.matmul(
                            psA, w_r[:, cb, 0:128], xT[:, cb, :],
                            start=(cb == 0), stop=(cb == CB - 1),
                        )
                    nc.vector.tensor_scalar_add(
                        dA[:, 512 * it : 512 * (it + 1)], psA, bc
                    )
                # q-tail (head 2 q, rows 0-63) + k-tail (head 2 k, rows 64-127)
                # in one accumulation; k half realigned to base 0 via DMA
                psB = psm.tile([128, 512], f32, tag="proj", bufs=2, name="psB")
                for cb in range(CB):
                    nc.tensor.matmul(
                        psB, wqk_t[:, cb, :], xT[:, cb, :],
                        start=(cb == 0), stop=(cb == CB - 1),
                    )
                nc.vector.tensor_scalar_add(
                    q2[:, 512 * it : 512 * (it + 1)], psB[0:64, :], bq_c2
                )
                ktmp = pm.tile([128, 512], f32r, tag="ktmp", bufs=2, name="ktmp")
                nc.vector.tensor_scalar_add(
                    ktmp[64:128, :], psB[64:128, :], bk_c2h[64:128, :]
                )
                nc.sync.dma_start(
                    out=k2[:, 512 * it : 512 * (it + 1)], in_=ktmp[64:128, :]
                )
                for tb in range(4):
                    psV = psm.tile([128, 256], f32, tag="proj", bufs=2, name="psV")
                    for cb in range(CB):
                        nc.tensor.matmul(
                            psV, xT[:, cb, 128 * tb : 128 * (tb + 1)], wv_r[:, cb, :],
                            start=(cb == 0), stop=(cb == CB - 1),
                        )
                    tk = 4 * it + tb
                    nc.vector.tensor_add(
                        v1[:, tk, :, 0:D],
                        psV[:, 0:MPC].rearrange("p (h d) -> p h d", h=HPC),
                        bv_b.rearrange("p (h d) -> p h d", h=HPC),
                    )

            def do_attn(h, iq):
                qh = (q01[0:64], q01[64:128], q2[0:64])[h]
                kh = (k01[0:64], k01[64:128], k2[0:64])[h]
                ps_o = psm.tile([65, 512], f32, tag="o", bufs=2, name="ps_o")
                qs = qh[:, 512 * iq : 512 * (iq + 1)]
                # full tk blocks in pairs: one [128, 1024] exp, no masking
                for p in range(2 * iq):
                    ik0, ik1 = 2 * p, 2 * p + 1
                    ps2 = psm.tile([128, 1024], f32, tag="s", bufs=2, name="ps2")
                    nc.tensor.matmul(
                        ps2[:, 0:512], kh[:, 128 * ik0 : 128 * (ik0 + 1)], qs,
                        start=True, stop=True,
                    )
                    nc.tensor.matmul(
                        ps2[:, 512:1024], kh[:, 128 * ik1 : 128 * (ik1 + 1)], qs,
                        start=True, stop=True,
                    )
                    pT = pm.tile([128, 1024], f32r, tag="pT", bufs=3, name="pT")
                    nc.scalar.activation(pT, ps2, EXP, scale=0.125)
                    nc.tensor.matmul(
                        ps_o, v1[:, ik0, h, :], pT[:, 0:512],
                        start=(ik0 == 0), stop=False,
                    )
                    nc.tensor.matmul(
                        ps_o, v1[:, ik1, h, :], pT[:, 512:1024],
                        start=False, stop=False,
                    )
                # diagonal region: 4 single blocks with causal masking
                for j in range(4):
                    ik = 4 * iq + j
                    col0 = 0 if j < 1 else (128 if j == 1 else 256)
                    ps2 = psm.tile([128, 1024], f32, tag="s", bufs=2, name="ps2")
                    nc.tensor.matmul(
                        ps2[:, col0:512],
                        kh[:, 128 * ik : 128 * (ik + 1)],
                        qh[:, 512 * iq + col0 : 512 * (iq + 1)],
                        start=True, stop=True,
                    )
                    pT = pm.tile([128, 1024], f32r, tag="pT", bufs=3, name="pT")
                    nc.scalar.activation(pT[:, col0:512], ps2[:, col0:512], EXP, scale=0.125)
                    if j == 3:
                        nc.vector.tensor_mul(pT[:, 256:512], pT[:, 256:512], mask)
                    else:
                        nc.vector.tensor_mul(
                            pT[:, col0 : col0 + 128],
                            pT[:, col0 : col0 + 128],
                            mask[:, 128:256],
                        )
                    nc.tensor.matmul(
                        ps_o[:, col0:], v1[:, ik, h, :], pT[:, col0:512],
                        start=(ik == 0), stop=(j == 3),
                    )
                recip = pm.tile([128, 512], f32, tag="rc", bufs=3, name="recip")
                nc.vector.reciprocal(recip[64:65, :], ps_o[64:65, :])
                rc_d = drm.tile([512], f32, tag="rcd", bufs=3, name="rc_d")
                nc.sync.dma_start(out=rc_d.unsqueeze(0), in_=recip[64:65, :])
                bcast = pm.tile([64, 512], f32, tag="bc", bufs=3, name="bcast")
                nc.sync.dma_start(
                    out=bcast,
                    in_=bass.AP(tensor=rc_d.tensor, offset=rc_d[:].offset, ap=[[0, 64]] + rc_d[:].ap),
                )
                att_n = pm.tile([64, 512], f32r, tag="an", bufs=3, name="att_n")
                nc.vector.tensor_mul(att_n, ps_o[0:64, :], bcast)
                nc.sync.dma_start(out=a2a_in[h][iq, :, :], in_=att_n)

            for iq in range(4):
                do_proj(iq)
                for h in range(HPC):
                    do_attn(h, iq)
            for h in range(HPC):
                for iq in range(4, NQB):
                    if h == 0:
                        do_proj(iq)
                    do_attn(h, iq)
                nc.gpsimd.collective_compute(
                    "AllToAll",
                    mybir.AluOpType.bypass,
                    replica_groups=[list(range(NCORES))],
                    ins=[a2a_in[h][:]],
                    outs=[a2a_out[h][:]],
                )

        # ---- phase 3: output projection -----------------------------------
        # gathered layout is head-major: flats[h] rows = 64*src + d; the host
        # permutes Wo's rows to match (see kernel()).
        flats = tuple(a.rearrange("s d t -> (s d) t") for a in a2a_out)  # [512, 512]
        with (
            tc.tile_pool(name="p3", bufs=1) as p3,
            tc.tile_pool(name="ps3", bufs=1, space="PSUM") as ps3,
        ):
            nc.sync.dma_start(out=wo_r, in_=wo_in.rearrange("(cb p) m -> p cb m", p=128))
            nc.sync.dma_start(
                out=bo_b,
                in_=bass.AP(tensor=bo_in.ap().tensor, offset=0, ap=[[0, 128]] + bo_in.ap().ap),
            )
            for bb in range(2):
                for tb in range(4):
                    ps_a = ps3.tile([128, 512], f32, tag="a", bufs=4)
                    ps_b = ps3.tile([128, 256], f32, tag="b", bufs=4)
                    for idx in range(CB):
                        h_l, half = divmod(idx, 2)
                        lt = p3.tile([128, 128], f32r, tag="ltr", bufs=12)
                        nc.sync.dma_start(
                            out=lt,
                            in_=flats[h_l][
                                256 * bb + 128 * half : 256 * bb + 128 * (half + 1),
                                128 * tb : 128 * (tb + 1),
                            ],
                        )
                        nc.tensor.matmul(
                            ps_a, lt, wo_r[:, idx, 0:512],
                            start=(idx == 0), stop=(idx == CB - 1),
                        )
                        nc.tensor.matmul(
                            ps_b, lt, wo_r[:, idx, 512:C],
                            start=(idx == 0), stop=(idx == CB - 1),
                        )
                    out_t = p3.tile([128, C], f32, tag="ot", bufs=3)
                    nc.vector.tensor_add(out_t[:, 0:512], ps_a, bo_b[:, 0:512])
                    nc.vector.tensor_add(out_t[:, 512:C], ps_b, bo_b[:, 512:C])
                    nc.sync.dma_start(
                        out=out_d[bb, 128 * tb : 128 * (tb + 1), :], in_=out_t
                    )

    nc.finalize()
    return nc


def kernel(x, Wq, bq, Wk, bk, Wv, bv, Wo, bo):
    if "nc" not in _CACHE:
        _CACHE["nc"] = _build()
    nc = _CACHE["nc"]

    x = np.asarray(x, dtype=np.float32)
    # permute Wo rows from global head-dim order (192g + 64h + d) to the
    # head-major gathered layout (256h + 64g + d) used by phase 3
    perm = np.empty(C, dtype=np.int64)
    for h_l in range(HPC):
        for g in range(4):
            perm[256 * h_l + 64 * g : 256 * h_l + 64 * g + 64] = np.arange(
                MPC * g + D * h_l, MPC * g + D * h_l + D
            )
    wo_send = np.ascontiguousarray(np.asarray(Wo, np.float32)[perm, :])
    in_maps = []
    for c in range(NCORES):
        b, g = c // 4, c % 4
        sl = slice(MPC * g, MPC * (g + 1))
        in_maps.append({
            "x": np.ascontiguousarray(x[b]),
            "wq": np.ascontiguousarray(np.asarray(Wq, np.float32)[:, sl]),
            "wk": np.ascontiguousarray(np.asarray(Wk, np.float32)[:, sl]),
            "wv": np.ascontiguousarray(np.asarray(Wv, np.float32)[:, sl]),
            "bq": np.ascontiguousarray(np.asarray(bq, np.float32)[sl]),
            "bk": np.ascontiguousarray(np.asarray(bk, np.float32)[sl]),
            "bv": np.ascontiguousarray(np.asarray(bv, np.float32)[sl]),
            "wo": wo_send,
            "bo": np.ascontiguousarray(np.asarray(bo, np.float32)),
        })

    res = run_bass_kernel_spmd(nc, in_maps, core_ids=list(range(NCORES)))
    out = np.empty((2, T, C), dtype=np.float32)
    for j in range(NCORES):
        r = res.results[j]["out"]
        out[0, 512 * j : 512 * (j + 1), :] = r[0]
        out[1, 512 * j : 512 * (j + 1), :] = r[1]
    return out
